# revision 13
# baseline (speedup 1.0000x reference)
"""Multi-head attention (16 heads, d_model=1024, bs=2, qlen=2048) on 8 trn2 cores.

Sharding: core c -> batch b = c//4, head-group r = c%4 (heads 4r..4r+3, i.e.
dims 256r..256r+256 of the head axis).  Each core projects q/k/v only for its
own 4 heads (Megatron column split), runs scores/softmax/AV for those heads,
then per-pair AllGathers of the per-core context within each batch group of 4
cores feed the row-split output projection (each core computes its own 256
output columns; no reduction needed).

Numerics: bf16 matmul operands, fp32 PSUM accumulation, softmax in fp32 on the
scalar engine.  Scores for the two heads of a pair are computed as two
concurrent row-tiled K=64 matmuls (head A in PE rows 0-63, head B in rows
64-127) sharing one streamed q tile, into one [128,1024] PSUM tile per k-chunk
(head A cols 0:512, head B 512:1024) so scores double-buffer in 4 PSUM banks.
V is stored interleaved with ones columns so one M=128 matmul per head
accumulates both context rows and softmax denominators.  1/sqrt(d) and q_b are
folded into q_w/q_b on the host; v_b is deferred past the softmax (rows of
P/sum sum to 1) and added to the normalized context.  The softmax denominator
reciprocal is computed on the 2 raw rows and broadcast across partitions with
a single K=2 matmul.
"""

import functools
import os
import sys

import numpy as np

for _p in ("/opt/trn_rl_repo", "/root/.axon_site/_ro/trn_rl_repo"):
    if os.path.isdir(_p) and _p not in sys.path:
        sys.path.append(_p)

import ml_dtypes

from concourse import bacc, bass, mybir, tile
from concourse.bass_utils import run_bass_kernel_spmd

BF16 = ml_dtypes.bfloat16
FP32 = mybir.dt.float32
BF16_DT = mybir.dt.bfloat16

N_CORES = 8
BS = 2
L = 2048  # sequence length
D = 1024  # model dim
DH = 64  # head dim
OWN = 256  # head dims per core (4 heads)
KC_D = 8  # 1024 / 128 contraction chunks for projections
NT = 4  # 2048 / 512 token tiles
KT = 16  # 2048 / 128 key-token chunks
WARMUP_MM = 10  # dummy matmuls to warm the PE clock gate during input DMA

LAST_EXEC_NS = None
LAST_RESULTS = None


def _build_nc(apply_mask: bool):
    nc = bacc.Bacc(None, num_devices=N_CORES)

    xT = nc.dram_tensor("xT", [D, L], BF16_DT, kind="ExternalInput")
    wq = nc.dram_tensor("wq", [D, OWN], BF16_DT, kind="ExternalInput")
    wk = nc.dram_tensor("wk", [D, OWN], BF16_DT, kind="ExternalInput")
    wv = nc.dram_tensor("wv", [D, OWN], BF16_DT, kind="ExternalInput")
    wo = nc.dram_tensor("wo", [D, OWN], BF16_DT, kind="ExternalInput")
    qb2 = nc.dram_tensor("qb2", [128, 2], FP32, kind="ExternalInput")
    kb2 = nc.dram_tensor("kb2", [128, 2], FP32, kind="ExternalInput")
    ob2 = nc.dram_tensor("ob2", [128, 2], FP32, kind="ExternalInput")
    mask01 = nc.dram_tensor("mask01", [128, KT], FP32, kind="ExternalInput")
    outT = nc.dram_tensor("outT", [OWN, L], BF16_DT, kind="ExternalOutput")

    Exp = mybir.ActivationFunctionType.Exp
    GRPS = [[0, 1, 2, 3], [4, 5, 6, 7]]

    with tile.TileContext(nc) as tc:
        with (
            tc.tile_pool(name="const", bufs=1) as const,
            tc.tile_pool(name="work", bufs=2) as work,
            tc.tile_pool(name="ps", bufs=1, space="PSUM") as ps,
            tc.tile_pool(name="dram", bufs=1, space="DRAM") as dram,
        ):
            # ---- stage inputs into SBUF (order matters: earliest consumers
            # first so the PE never waits on a transfer it could have had) ----
            wk_sb = []
            for i in range(KC_D):
                t = const.tile([128, OWN], BF16_DT, tag=f"wk{i}", name=f"wk_sb{i}")
                nc.sync.dma_start(t, wk[i * 128 : (i + 1) * 128, :])
                wk_sb.append(t)

            def load_small(dram_t, nm, cols):
                t = const.tile([128, cols], FP32, tag=nm, name=f"{nm}_sb")
                nc.sync.dma_start(t, dram_t[:, :])
                return t

            qb_sb = load_small(qb2, "qb", 2)
            kb_sb = load_small(kb2, "kb", 2)
            ob_sb = load_small(ob2, "ob", 2)
            mask_sb = load_small(mask01, "mask", KT) if apply_mask else None

            x_sb = [
                const.tile([128, L], BF16_DT, tag=f"x{i}", name=f"x_sb{i}")
                for i in range(KC_D)
            ]

            def load_x(n):
                for i in range(KC_D):
                    eng = nc.sync if i % 2 == 0 else nc.gpsimd
                    eng.dma_start(
                        x_sb[i][:, n * 512 : (n + 1) * 512],
                        xT[i * 128 : (i + 1) * 128, n * 512 : (n + 1) * 512],
                    )

            load_x(0)
            load_x(1)

            def load_w(dram_t, nm):
                tiles = []
                for i in range(KC_D):
                    t = const.tile([128, OWN], BF16_DT, tag=f"{nm}{i}", name=f"{nm}_sb{i}")
                    nc.sync.dma_start(t, dram_t[i * 128 : (i + 1) * 128, :])
                    tiles.append(t)
                return tiles

            wv_sb = load_w(wv, "wv")
            load_x(2)
            load_x(3)
            wq_sb = load_w(wq, "wq")
            wo_sb = load_w(wo, "wo")

            ones_sb = const.tile([128, DH], BF16_DT, tag="ones", name="ones_sb")
            nc.vector.memset(ones_sb, 1.0)

            # eb: selector for the denominator-reciprocal broadcast.  Row 64
            # carries 1/denom_h0 (dup'd rows 64-127 of recipf), row 0 carries
            # 1/denom_h1; rb = eb.T @ recipf puts 1/denom_h0 on partitions
            # 0-63 and 1/denom_h1 on 64-127, next to their context rows.
            eb_sb = const.tile([128, 128], BF16_DT, tag="eb", name="eb_sb")
            nc.vector.memset(eb_sb, 0.0)
            nc.vector.memset(eb_sb[64:65, 0:64], 1.0)
            nc.vector.memset(eb_sb[0:1, 64:128], 1.0)

            # Warm the PE clock gate while inputs stream in: dependency-free
            # matmuls keep the HAM busy-window active.
            wsc = const.tile([128, 512], BF16_DT, tag="wsc", name="wsc")
            nc.vector.memset(wsc, 0.0)
            for _ in range(WARMUP_MM):
                wps = ps.tile([128, 512], FP32, tag="op", bufs=2, name="wps")
                nc.tensor.matmul(wps[0:64, :], lhsT=ones_sb[:, 0:64], rhs=wsc)

            # Warm the collective path: tiny AllGather absorbs the CC-core
            # first-collective overhead while the projections run.
            wag_in = dram.tile([128, 2], FP32, tag="wagi", name="wagi")
            wag_out = dram.tile([512, 2], FP32, tag="wago", name="wago")
            nc.sync.dma_start(wag_in[:, :], qb_sb)
            nc.gpsimd.collective_compute(
                "AllGather",
                mybir.AluOpType.bypass,
                replica_groups=GRPS,
                ins=[wag_in.opt()],
                outs=[wag_out.opt()],
            )

            # ---- k projection.  kT2[p] holds the pair's two heads packed:
            # rows 0-63 = head 2p dims, rows 64-127 = head 2p+1 dims. ----
            kT2 = [
                const.tile([128, L], BF16_DT, tag=f"kT{p}", name=f"kT2_{p}")
                for p in range(2)
            ]

            def kproj(n, m):
                nsl = slice(n * 512, (n + 1) * 512)
                pp = ps.tile([128, 512], FP32, tag="sc", bufs=2, name="projk_ps")
                for kc in range(KC_D):
                    nc.tensor.matmul(
                        pp,
                        lhsT=wk_sb[kc][:, m * 128 : (m + 1) * 128],
                        rhs=x_sb[kc][:, nsl],
                        start=(kc == 0),
                        stop=(kc == KC_D - 1),
                    )
                nc.vector.tensor_scalar_add(
                    kT2[m][:, nsl], pp, kb_sb[:, m : m + 1]
                )

            # ---- v projection (tokens on partitions; no bias).  Stored
            # interleaved with ones columns so a single M=128 matmul per head
            # produces context rows and denominator rows: per pair block of
            # 256 cols: [v_h0 | ones | ones | v_h1]. ----
            v_sb = [
                const.tile([128, 512], BF16_DT, tag=f"v{t}", name=f"v_sb{t}")
                for t in range(KT)
            ]

            def vproj(t):
                nc.vector.memset(v_sb[t][:, 64:192], 1.0)
                nc.vector.memset(v_sb[t][:, 320:448], 1.0)
                pv = ps.tile([128, OWN], FP32, tag="sc", bufs=2, name="v_ps")
                for kc in range(KC_D):
                    nc.tensor.matmul(
                        pv,
                        lhsT=x_sb[kc][:, t * 128 : (t + 1) * 128],
                        rhs=wv_sb[kc],
                        start=(kc == 0),
                        stop=(kc == KC_D - 1),
                    )
                nc.vector.tensor_copy(v_sb[t][:, 0:64], pv[:, 0:64])
                nc.vector.tensor_copy(v_sb[t][:, 192:256], pv[:, 64:128])
                nc.vector.tensor_copy(v_sb[t][:, 256:320], pv[:, 128:192])
                nc.vector.tensor_copy(v_sb[t][:, 448:512], pv[:, 192:256])

            qT_sb = [
                const.tile([128, L], BF16_DT, tag=f"qT{p}", name=f"qT_sb{p}")
                for p in range(2)
            ]

            def qproj_pieces(qt):
                nsl = slice(qt * 512, (qt + 1) * 512)
                state = {}

                def piece(m, lo, hi):
                    def fn():
                        if lo == 0:
                            state[m] = ps.tile(
                                [128, 512], FP32, tag="op", bufs=2, name="projq_ps"
                            )
                        pp = state[m]
                        for kc in range(lo, hi):
                            nc.tensor.matmul(
                                pp,
                                lhsT=wq_sb[kc][:, m * 128 : (m + 1) * 128],
                                rhs=x_sb[kc][:, nsl],
                                start=(kc == 0),
                                stop=(kc == KC_D - 1),
                            )
                        if hi == KC_D:
                            nc.vector.tensor_scalar_add(
                                qT_sb[m][:, nsl], pp, qb_sb[:, m : m + 1]
                            )
                    return fn

                return [piece(m, lo, lo + 4) for m in (0, 1) for lo in (0, 4)]

            # phase 0: k projection, most of v, q for tile 0.  Later v chunks
            # and pair-1 k chunks for the back half ride inside unit (0,0).
            for n in range(2):
                kproj(n, 0)
                kproj(n, 1)
            kproj(2, 0)
            kproj(3, 0)
            for t in range(10):
                vproj(t)
            for fn in qproj_pieces(0):
                fn()

            # ---- attention ----
            ctx_sb = [
                const.tile([128, L], BF16_DT, tag=f"ctx{p}", name=f"ctx_sb{p}")
                for p in range(2)
            ]
            ag_out = [[None, None] for _ in range(NT)]

            def normalize(qt, p, cs0, cs1):
                # cs0: rows 0-63 ctx_h0, rows 64-127 denominators (x64)
                # cs1: rows 0-63 denominators (x64), rows 64-127 ctx_h1
                # Copy the duplicated denominator rows to SBUF, swap the head
                # halves with one eb matmul (DVE cannot shift partitions), and
                # take reciprocals of the broadcast result.
                qsl = slice(qt * 512, (qt + 1) * 512)
                denb = work.tile([128, 512], BF16_DT, tag="denb", name="denb")
                nc.vector.tensor_copy(denb[64:128, :], cs0[64:128, :])
                nc.vector.tensor_copy(denb[0:64, :], cs1[0:64, :])
                rb = ps.tile([128, 512], FP32, tag="op", bufs=2, name="rb")
                nc.tensor.matmul(rb, lhsT=eb_sb, rhs=denb)
                recipf = work.tile([128, 512], FP32, tag="recipf", name="recipf")
                nc.vector.reciprocal_approx_fast(recipf, rb)
                # v_b is folded into o_b on the host (rows of P/sum sum to 1),
                # so the normalized context needs no bias add.
                nc.vector.tensor_mul(
                    ctx_sb[p][0:64, qsl], cs0[0:64, :], recipf[0:64, :]
                )
                nc.vector.tensor_mul(
                    ctx_sb[p][64:128, qsl], cs1[64:128, :], recipf[64:128, :]
                )
                # AllGather this pair's context slice within the batch group.
                ag_in = dram.tile(
                    [128, 512], BF16_DT, tag=f"agi{qt}{p}", name=f"agi{qt}{p}"
                )
                ago = dram.tile(
                    [512, 512], BF16_DT, tag=f"ago{qt}{p}", name=f"ago{qt}{p}"
                )
                ag_out[qt][p] = ago
                nc.sync.dma_start(ag_in[:, :], ctx_sb[p][:, qsl])
                nc.gpsimd.collective_compute(
                    "AllGather",
                    mybir.AluOpType.bypass,
                    replica_groups=GRPS,
                    ins=[ag_in.opt()],
                    outs=[ago.opt()],
                )

            def oproj_pieces(qt):
                # output projection for token tile qt (own 256 columns).
                # Chunk kc covers global ctx dims [128kc,128kc+128) = rank
                # kc//2, pair kc%2.  Pair-0 chunks are listed first so the
                # accumulation starts before the pair-1 gather lands.
                qsl = slice(qt * 512, (qt + 1) * 512)
                state = {}
                srcs = []
                for pp in range(2):
                    for r in range(4):
                        srcs.append((2 * r + pp, pp, r))

                def load_cf(pp):
                    def fn():
                        for i, (kc, ppi, r) in enumerate(srcs):
                            if ppi != pp:
                                continue
                            t = work.tile(
                                [128, 512], BF16_DT, tag=f"cf{i}", name=f"cf{i}"
                            )
                            nc.sync.dma_start(
                                t, ag_out[qt][ppi][r * 128 : (r + 1) * 128, :]
                            )
                            state[i] = t
                    return fn

                def mm_piece(m, lo, hi):
                    def fn():
                        if lo == 0:
                            state[f"po{m}"] = ps.tile(
                                [128, 512], FP32, tag="op", bufs=2, name="o_ps"
                            )
                        po = state[f"po{m}"]
                        for i in range(lo, hi):
                            kc, ppi, r = srcs[i]
                            nc.tensor.matmul(
                                po,
                                lhsT=wo_sb[kc][:, m * 128 : (m + 1) * 128],
                                rhs=state[i],
                                start=(i == 0),
                                stop=(i == KC_D - 1),
                            )
                        if hi == KC_D:
                            osb = work.tile(
                                [128, 512], BF16_DT, tag="osb", name="osb"
                            )
                            nc.vector.tensor_scalar_add(osb, po, ob_sb[:, m : m + 1])
                            nc.sync.dma_start(
                                outT[m * 128 : (m + 1) * 128, qsl], osb
                            )
                    return fn

                return [
                    load_cf(0),
                    mm_piece(0, 0, 4),
                    mm_piece(1, 0, 4),
                    load_cf(1),
                    mm_piece(0, 4, 8),
                    mm_piece(1, 4, 8),
                ]

            units = [(qt, p) for qt in range(NT) for p in range(2)]
            deferred = [
                functools.partial(vproj, t) for t in range(10, KT)
            ] + [
                functools.partial(kproj, 2, 1),
                functools.partial(kproj, 3, 1),
            ]

            for qt, p in units:
                qsl = slice(qt * 512, (qt + 1) * 512)
                cs0 = ps.tile([128, 512], FP32, tag="ctx", bufs=1, name="cs0")
                cs1 = ps.tile([128, 512], FP32, tag="sums", bufs=1, name="cs1")

                def av(kc, pr, cs0=cs0, cs1=cs1, p=p):
                    st = kc == 0
                    sp = kc == KT - 1
                    nc.tensor.matmul(
                        cs0,
                        lhsT=v_sb[kc][:, p * 256 : p * 256 + 128],
                        rhs=pr[:, 0:512],
                        start=st,
                        stop=sp,
                    )
                    nc.tensor.matmul(
                        cs1,
                        lhsT=v_sb[kc][:, p * 256 + 128 : p * 256 + 256],
                        rhs=pr[:, 512:1024],
                        start=st,
                        stop=sp,
                    )

                first_unit = qt == 0 and p == 0
                prev = None
                for kc in range(KT):
                    s01 = ps.tile([128, 1024], FP32, tag="sc", bufs=2, name="s01")
                    nc.tensor.matmul(
                        s01[:, 0:512],
                        lhsT=kT2[p][0:64, kc * 128 : (kc + 1) * 128],
                        rhs=qT_sb[p][0:64, qsl],
                    )
                    nc.tensor.matmul(
                        s01[:, 512:1024],
                        lhsT=kT2[p][64:128, kc * 128 : (kc + 1) * 128],
                        rhs=qT_sb[p][64:128, qsl],
                    )
                    pr = work.tile([128, 1024], BF16_DT, tag="pr", name="pr")
                    if apply_mask:
                        e01 = work.tile([128, 1024], FP32, tag="e01", name="e01")
                        nc.scalar.activation(e01, s01, Exp)
                        nc.vector.tensor_scalar_mul(
                            pr[:, 0:512], e01[:, 0:512], mask_sb[:, kc : kc + 1]
                        )
                        nc.vector.tensor_scalar_mul(
                            pr[:, 512:1024], e01[:, 512:1024], mask_sb[:, kc : kc + 1]
                        )
                    else:
                        nc.scalar.activation(pr, s01, Exp)
                    # trailing work (projections for neighbouring tiles, the
                    # previous tile's output projection) trickles in between
                    # score groups, keeping the tensor stream dense
                    if deferred and kc >= 2:
                        deferred.pop(0)()
                    if prev is not None:
                        av(kc - 1, prev)
                    prev = pr
                av(KT - 1, prev)
                while deferred:
                    deferred.pop(0)()
                normalize(qt, p, cs0, cs1)

                # schedule trailing work for the NEXT unit:
                #   unit (qt,1) runs oproj(qt-1) (qproj(1) for the first one),
                #   unit (qt+1,0) runs qproj(qt+2).
                if p == 0:
                    deferred = oproj_pieces(qt - 1) if qt > 0 else qproj_pieces(1)
                else:
                    deferred = qproj_pieces(qt + 2) if qt + 2 < NT else []

            for fn in oproj_pieces(NT - 1):
                fn()

    nc.finalize()
    return nc


@functools.lru_cache(maxsize=2)
def _built(apply_mask: bool):
    return _build_nc(apply_mask)


def kernel(input, mask, q_w, q_b, k_w, k_b, v_w, v_b, o_w, o_b):
    global LAST_EXEC_NS, LAST_RESULTS
    input = np.asarray(input, dtype=np.float32)
    mask = np.asarray(mask)
    apply_mask = not bool(np.all(mask != 0))
    nc = _built(apply_mask)

    qw = (np.asarray(q_w, np.float32) / 8.0).astype(BF16)
    kw = np.asarray(k_w, np.float32).astype(BF16)
    vw = np.asarray(v_w, np.float32).astype(BF16)
    ow = np.asarray(o_w, np.float32).astype(BF16)
    qb = np.asarray(q_b, np.float32) / 8.0
    kb = np.asarray(k_b, np.float32)
    ob = (
        np.asarray(o_b, np.float64)
        + np.asarray(o_w, np.float64) @ np.asarray(v_b, np.float64)
    ).astype(np.float32)

    in_maps = []
    for c in range(N_CORES):
        b, r = divmod(c, 4)
        own = slice(OWN * r, OWN * (r + 1))
        m01 = (mask[b] != 0).astype(np.float32)
        in_maps.append(
            {
                "xT": np.ascontiguousarray(input[b].T.astype(BF16)),
                "wq": np.ascontiguousarray(qw[own, :].T),
                "wk": np.ascontiguousarray(kw[own, :].T),
                "wv": np.ascontiguousarray(vw[own, :].T),
                "wo": np.ascontiguousarray(ow[own, :].T),
                "qb2": np.ascontiguousarray(qb[own].reshape(2, 128).T),
                "kb2": np.ascontiguousarray(kb[own].reshape(2, 128).T),
                "ob2": np.ascontiguousarray(ob[own].reshape(2, 128).T),
                "mask01": np.ascontiguousarray(m01.reshape(KT, 128).T),
            }
        )

    trace = os.environ.get("KERNEL_TRACE", "0") == "1"
    res = run_bass_kernel_spmd(
        nc,
        in_maps,
        core_ids=list(range(N_CORES)),
        trace=trace,
        trace_cores=list(range(N_CORES)) if trace else None,
        stitch_traces=False,
    )
    LAST_EXEC_NS = res.exec_time_ns
    LAST_RESULTS = res

    out = np.empty((BS, L, D), dtype=np.float32)
    for c in range(N_CORES):
        b, r = divmod(c, 4)
        out[b, :, OWN * r : OWN * (r + 1)] = res.results[c]["outT"].T.astype(
            np.float32
        )
    return out


# revision 14
# speedup vs baseline: 1.0375x; 1.0375x over previous
"""Multi-head attention (16 heads, d_model=1024, bs=2, qlen=2048) on 8 trn2 cores.

Sharding: core c -> batch b = c//4, head-group r = c%4 (heads 4r..4r+3, i.e.
dims 256r..256r+256 of the head axis).  Each core projects q/k/v only for its
own 4 heads (Megatron column split), runs scores/softmax/AV for those heads,
then per-pair AllGathers of the per-core context within each batch group of 4
cores feed the row-split output projection (each core computes its own 256
output columns; no reduction needed).

Numerics: bf16 matmul operands, fp32 PSUM accumulation, softmax in fp32 on the
scalar engine.  Scores for the two heads of a pair are computed as two
concurrent row-tiled K=64 matmuls (head A in PE rows 0-63, head B in rows
64-127) sharing one streamed q tile, into one [128,1024] PSUM tile per k-chunk
(head A cols 0:512, head B 512:1024) so scores double-buffer in 4 PSUM banks.
V is stored interleaved with ones columns so one M=128 matmul per head
accumulates both context rows and softmax denominators.  1/sqrt(d) and q_b are
folded into q_w/q_b on the host; v_b is deferred past the softmax (rows of
P/sum sum to 1) and added to the normalized context.  The softmax denominator
reciprocal is computed on the 2 raw rows and broadcast across partitions with
a single K=2 matmul.
"""

import functools
import os
import sys

import numpy as np

for _p in ("/opt/trn_rl_repo", "/root/.axon_site/_ro/trn_rl_repo"):
    if os.path.isdir(_p) and _p not in sys.path:
        sys.path.append(_p)

import ml_dtypes

from concourse import bacc, bass, mybir, tile
from concourse.bass_utils import run_bass_kernel_spmd

BF16 = ml_dtypes.bfloat16
FP32 = mybir.dt.float32
BF16_DT = mybir.dt.bfloat16

N_CORES = 8
BS = 2
L = 2048  # sequence length
D = 1024  # model dim
DH = 64  # head dim
OWN = 256  # head dims per core (4 heads)
KC_D = 8  # 1024 / 128 contraction chunks for projections
NT = 4  # 2048 / 512 token tiles
KT = 16  # 2048 / 128 key-token chunks
WARMUP_MM = 10  # dummy matmuls to warm the PE clock gate during input DMA

LAST_EXEC_NS = None
LAST_RESULTS = None


def _build_nc(apply_mask: bool):
    nc = bacc.Bacc(None, num_devices=N_CORES)

    xT = nc.dram_tensor("xT", [D, L], BF16_DT, kind="ExternalInput")
    wq = nc.dram_tensor("wq", [D, OWN], BF16_DT, kind="ExternalInput")
    wk = nc.dram_tensor("wk", [D, OWN], BF16_DT, kind="ExternalInput")
    wv = nc.dram_tensor("wv", [D, OWN], BF16_DT, kind="ExternalInput")
    wo = nc.dram_tensor("wo", [D, OWN], BF16_DT, kind="ExternalInput")
    qb2 = nc.dram_tensor("qb2", [128, 2], FP32, kind="ExternalInput")
    kb2 = nc.dram_tensor("kb2", [128, 2], FP32, kind="ExternalInput")
    ob2 = nc.dram_tensor("ob2", [128, 2], FP32, kind="ExternalInput")
    mask01 = nc.dram_tensor("mask01", [128, KT], FP32, kind="ExternalInput")
    outT = nc.dram_tensor("outT", [OWN, L], BF16_DT, kind="ExternalOutput")

    Exp = mybir.ActivationFunctionType.Exp
    GRPS = [[0, 1, 2, 3], [4, 5, 6, 7]]

    with tile.TileContext(nc) as tc:
        with (
            tc.tile_pool(name="const", bufs=1) as const,
            tc.tile_pool(name="work", bufs=2) as work,
            tc.tile_pool(name="ps", bufs=1, space="PSUM") as ps,
            tc.tile_pool(name="dram", bufs=1, space="DRAM") as dram,
        ):
            # ---- stage inputs into SBUF (order matters: earliest consumers
            # first so the PE never waits on a transfer it could have had) ----
            wk_sb = []
            for i in range(KC_D):
                t = const.tile([128, OWN], BF16_DT, tag=f"wk{i}", name=f"wk_sb{i}")
                nc.sync.dma_start(t, wk[i * 128 : (i + 1) * 128, :])
                wk_sb.append(t)

            def load_small(dram_t, nm, cols):
                t = const.tile([128, cols], FP32, tag=nm, name=f"{nm}_sb")
                nc.sync.dma_start(t, dram_t[:, :])
                return t

            qb_sb = load_small(qb2, "qb", 2)
            kb_sb = load_small(kb2, "kb", 2)
            ob_sb = load_small(ob2, "ob", 2)
            mask_sb = load_small(mask01, "mask", KT) if apply_mask else None

            x_sb = [
                const.tile([128, L], BF16_DT, tag=f"x{i}", name=f"x_sb{i}")
                for i in range(KC_D)
            ]

            def load_x(n):
                for i in range(KC_D):
                    eng = nc.sync if i % 2 == 0 else nc.gpsimd
                    eng.dma_start(
                        x_sb[i][:, n * 512 : (n + 1) * 512],
                        xT[i * 128 : (i + 1) * 128, n * 512 : (n + 1) * 512],
                    )

            load_x(0)
            load_x(1)

            def load_w(dram_t, nm):
                tiles = []
                for i in range(KC_D):
                    t = const.tile([128, OWN], BF16_DT, tag=f"{nm}{i}", name=f"{nm}_sb{i}")
                    nc.sync.dma_start(t, dram_t[i * 128 : (i + 1) * 128, :])
                    tiles.append(t)
                return tiles

            wv_sb = load_w(wv, "wv")
            load_x(2)
            load_x(3)
            wq_sb = load_w(wq, "wq")
            wo_sb = load_w(wo, "wo")

            ones_sb = const.tile([128, DH], BF16_DT, tag="ones", name="ones_sb")
            nc.vector.memset(ones_sb, 1.0)

            # eb: selector for the denominator-reciprocal broadcast.  Row 64
            # carries 1/denom_h0 (dup'd rows 64-127 of recipf), row 0 carries
            # 1/denom_h1; rb = eb.T @ recipf puts 1/denom_h0 on partitions
            # 0-63 and 1/denom_h1 on 64-127, next to their context rows.
            eb_sb = const.tile([128, 128], BF16_DT, tag="eb", name="eb_sb")
            nc.vector.memset(eb_sb, 0.0)
            nc.vector.memset(eb_sb[64:65, 0:64], 1.0)
            nc.vector.memset(eb_sb[0:1, 64:128], 1.0)

            # Warm the PE clock gate while inputs stream in: dependency-free
            # matmuls keep the HAM busy-window active.
            wsc = const.tile([128, 512], BF16_DT, tag="wsc", name="wsc")
            nc.vector.memset(wsc, 0.0)
            for _ in range(WARMUP_MM):
                wps = ps.tile([128, 512], FP32, tag="op", bufs=2, name="wps")
                nc.tensor.matmul(wps[0:64, :], lhsT=ones_sb[:, 0:64], rhs=wsc)

            # Warm the collective path: tiny AllGather absorbs the CC-core
            # first-collective overhead while the projections run.
            wag_in = dram.tile([128, 2], FP32, tag="wagi", name="wagi")
            wag_out = dram.tile([512, 2], FP32, tag="wago", name="wago")
            nc.sync.dma_start(wag_in[:, :], qb_sb)
            nc.gpsimd.collective_compute(
                "AllGather",
                mybir.AluOpType.bypass,
                replica_groups=GRPS,
                ins=[wag_in.opt()],
                outs=[wag_out.opt()],
            )

            # ---- k projection.  kT2[p] holds the pair's two heads packed:
            # rows 0-63 = head 2p dims, rows 64-127 = head 2p+1 dims. ----
            kT2 = [
                const.tile([128, L], BF16_DT, tag=f"kT{p}", name=f"kT2_{p}")
                for p in range(2)
            ]

            def kproj(n, m):
                nsl = slice(n * 512, (n + 1) * 512)
                pp = ps.tile([128, 512], FP32, tag="sc", bufs=2, name="projk_ps")
                for kc in range(KC_D):
                    nc.tensor.matmul(
                        pp,
                        lhsT=wk_sb[kc][:, m * 128 : (m + 1) * 128],
                        rhs=x_sb[kc][:, nsl],
                        start=(kc == 0),
                        stop=(kc == KC_D - 1),
                    )
                nc.vector.tensor_scalar_add(
                    kT2[m][:, nsl], pp, kb_sb[:, m : m + 1]
                )

            # ---- v projection (tokens on partitions; no bias).  Stored
            # interleaved with ones columns so a single M=128 matmul per head
            # produces context rows and denominator rows: per pair block of
            # 256 cols: [v_h0 | ones | ones | v_h1]. ----
            v_sb = [
                const.tile([128, 512], BF16_DT, tag=f"v{t}", name=f"v_sb{t}")
                for t in range(KT)
            ]

            def vproj(t):
                nc.vector.memset(v_sb[t][:, 64:192], 1.0)
                nc.vector.memset(v_sb[t][:, 320:448], 1.0)
                pv = ps.tile([128, OWN], FP32, tag="sc", bufs=2, name="v_ps")
                for kc in range(KC_D):
                    nc.tensor.matmul(
                        pv,
                        lhsT=x_sb[kc][:, t * 128 : (t + 1) * 128],
                        rhs=wv_sb[kc],
                        start=(kc == 0),
                        stop=(kc == KC_D - 1),
                    )
                nc.vector.tensor_copy(v_sb[t][:, 0:64], pv[:, 0:64])
                nc.vector.tensor_copy(v_sb[t][:, 192:256], pv[:, 64:128])
                nc.vector.tensor_copy(v_sb[t][:, 256:320], pv[:, 128:192])
                nc.vector.tensor_copy(v_sb[t][:, 448:512], pv[:, 192:256])

            qT_sb = [
                const.tile([128, L], BF16_DT, tag=f"qT{p}", name=f"qT_sb{p}")
                for p in range(2)
            ]

            def qproj_pieces(qt):
                nsl = slice(qt * 512, (qt + 1) * 512)
                state = {}

                def piece(m, lo, hi):
                    def fn():
                        if lo == 0:
                            state[m] = ps.tile(
                                [128, 512], FP32, tag="op", bufs=2, name="projq_ps"
                            )
                        pp = state[m]
                        for kc in range(lo, hi):
                            nc.tensor.matmul(
                                pp,
                                lhsT=wq_sb[kc][:, m * 128 : (m + 1) * 128],
                                rhs=x_sb[kc][:, nsl],
                                start=(kc == 0),
                                stop=(kc == KC_D - 1),
                            )
                        if hi == KC_D:
                            nc.vector.tensor_scalar_add(
                                qT_sb[m][:, nsl], pp, qb_sb[:, m : m + 1]
                            )
                    return fn

                return [piece(m, lo, lo + 4) for m in (0, 1) for lo in (0, 4)]

            # phase 0: k projection, most of v, q for tile 0.  Later v chunks
            # and pair-1 k chunks for the back half ride inside unit (0,0).
            for n in range(2):
                kproj(n, 0)
                kproj(n, 1)
            kproj(2, 0)
            kproj(3, 0)
            for t in range(10):
                vproj(t)
            for fn in qproj_pieces(0):
                fn()

            # ---- attention ----
            ctx_sb = [
                const.tile([128, L], BF16_DT, tag=f"ctx{p}", name=f"ctx_sb{p}")
                for p in range(2)
            ]
            ag_out = [[None, None] for _ in range(NT)]

            def normalize(qt, p, cs0, cs1):
                # cs0: rows 0-63 ctx_h0, rows 64-127 denominators (x64)
                # cs1: rows 0-63 denominators (x64), rows 64-127 ctx_h1
                # Copy the duplicated denominator rows to SBUF, swap the head
                # halves with one eb matmul (DVE cannot shift partitions), and
                # take reciprocals of the broadcast result.
                qsl = slice(qt * 512, (qt + 1) * 512)
                denb = work.tile([128, 512], BF16_DT, tag="denb", name="denb")
                nc.vector.tensor_copy(denb[64:128, :], cs0[64:128, :])
                nc.vector.tensor_copy(denb[0:64, :], cs1[0:64, :])
                rb = ps.tile([128, 512], FP32, tag="op", bufs=2, name="rb")
                nc.tensor.matmul(rb, lhsT=eb_sb, rhs=denb)
                recipf = work.tile([128, 512], FP32, tag="recipf", name="recipf")
                nc.vector.reciprocal_approx_fast(recipf, rb)
                # v_b is folded into o_b on the host (rows of P/sum sum to 1),
                # so the normalized context needs no bias add.
                nc.vector.tensor_mul(
                    ctx_sb[p][0:64, qsl], cs0[0:64, :], recipf[0:64, :]
                )
                nc.vector.tensor_mul(
                    ctx_sb[p][64:128, qsl], cs1[64:128, :], recipf[64:128, :]
                )
                # AllGather this pair's context slice within the batch group.
                ag_in = dram.tile(
                    [128, 512], BF16_DT, tag=f"agi{qt}{p}", name=f"agi{qt}{p}"
                )
                ago = dram.tile(
                    [512, 512], BF16_DT, tag=f"ago{qt}{p}", name=f"ago{qt}{p}"
                )
                ag_out[qt][p] = ago
                nc.sync.dma_start(ag_in[:, :], ctx_sb[p][:, qsl])
                nc.gpsimd.collective_compute(
                    "AllGather",
                    mybir.AluOpType.bypass,
                    replica_groups=GRPS,
                    ins=[ag_in.opt()],
                    outs=[ago.opt()],
                )

            def oproj_pieces(qt):
                # output projection for token tile qt (own 256 columns).
                # Chunk kc covers global ctx dims [128kc,128kc+128) = rank
                # kc//2, pair kc%2.  Pair-0 chunks are listed first so the
                # accumulation starts before the pair-1 gather lands.
                qsl = slice(qt * 512, (qt + 1) * 512)
                state = {}
                srcs = []
                for pp in range(2):
                    for r in range(4):
                        srcs.append((2 * r + pp, pp, r))

                def load_cf(pp):
                    def fn():
                        for i, (kc, ppi, r) in enumerate(srcs):
                            if ppi != pp:
                                continue
                            t = work.tile(
                                [128, 512], BF16_DT, tag=f"cf{i}", name=f"cf{i}"
                            )
                            nc.sync.dma_start(
                                t, ag_out[qt][ppi][r * 128 : (r + 1) * 128, :]
                            )
                            state[i] = t
                    return fn

                def mm_piece(m, lo, hi):
                    def fn():
                        if lo == 0:
                            state[f"po{m}"] = ps.tile(
                                [128, 512], FP32, tag="op", bufs=2, name="o_ps"
                            )
                        po = state[f"po{m}"]
                        for i in range(lo, hi):
                            kc, ppi, r = srcs[i]
                            nc.tensor.matmul(
                                po,
                                lhsT=wo_sb[kc][:, m * 128 : (m + 1) * 128],
                                rhs=state[i],
                                start=(i == 0),
                                stop=(i == KC_D - 1),
                            )
                        if hi == KC_D:
                            osb = work.tile(
                                [128, 512], BF16_DT, tag="osb", name="osb"
                            )
                            nc.vector.tensor_scalar_add(osb, po, ob_sb[:, m : m + 1])
                            nc.sync.dma_start(
                                outT[m * 128 : (m + 1) * 128, qsl], osb
                            )
                    return fn

                return [
                    load_cf(0),
                    mm_piece(0, 0, 4),
                    mm_piece(1, 0, 4),
                    load_cf(1),
                    mm_piece(0, 4, 8),
                    mm_piece(1, 4, 8),
                ]

            units = [(qt, p) for qt in range(NT) for p in range(2)]
            deferred = [
                functools.partial(vproj, t) for t in range(10, KT)
            ] + [
                functools.partial(kproj, 2, 1),
                functools.partial(kproj, 3, 1),
            ]

            for qt, p in units:
                qsl = slice(qt * 512, (qt + 1) * 512)
                cs0 = ps.tile([128, 512], FP32, tag="ctx", bufs=1, name="cs0")
                cs1 = ps.tile([128, 512], FP32, tag="sums", bufs=1, name="cs1")

                def av(kc, pr, cs0=cs0, cs1=cs1, p=p):
                    st = kc == 0
                    sp = kc == KT - 1
                    nc.tensor.matmul(
                        cs0,
                        lhsT=v_sb[kc][:, p * 256 : p * 256 + 128],
                        rhs=pr[:, 0:512],
                        start=st,
                        stop=sp,
                    )
                    nc.tensor.matmul(
                        cs1,
                        lhsT=v_sb[kc][:, p * 256 + 128 : p * 256 + 256],
                        rhs=pr[:, 512:1024],
                        start=st,
                        stop=sp,
                    )

                first_unit = qt == 0 and p == 0
                prev = None
                for kc in range(KT):
                    s01 = ps.tile([128, 1024], FP32, tag="sc", bufs=2, name="s01")
                    nc.tensor.matmul(
                        s01[:, 0:512],
                        lhsT=kT2[p][0:64, kc * 128 : (kc + 1) * 128],
                        rhs=qT_sb[p][0:64, qsl],
                    )
                    nc.tensor.matmul(
                        s01[:, 512:1024],
                        lhsT=kT2[p][64:128, kc * 128 : (kc + 1) * 128],
                        rhs=qT_sb[p][64:128, qsl],
                    )
                    pr = work.tile([128, 1024], BF16_DT, tag="pr", name="pr")
                    if apply_mask:
                        e01 = work.tile([128, 1024], FP32, tag="e01", name="e01")
                        nc.scalar.activation(e01, s01, Exp)
                        nc.vector.tensor_scalar_mul(
                            pr[:, 0:512], e01[:, 0:512], mask_sb[:, kc : kc + 1]
                        )
                        nc.vector.tensor_scalar_mul(
                            pr[:, 512:1024], e01[:, 512:1024], mask_sb[:, kc : kc + 1]
                        )
                    else:
                        nc.scalar.activation(pr, s01, Exp)
                    # trailing work (projections for neighbouring tiles, the
                    # previous tile's output projection) trickles in between
                    # score groups, keeping the tensor stream dense
                    if deferred and kc >= 2 and (first_unit or kc % 2 == 0):
                        deferred.pop(0)()
                    if prev is not None:
                        av(kc - 1, prev)
                    prev = pr
                av(KT - 1, prev)
                while deferred:
                    deferred.pop(0)()
                normalize(qt, p, cs0, cs1)

                # schedule trailing work for the NEXT unit:
                #   unit (qt,1) runs oproj(qt-1) (qproj(1) for the first one),
                #   unit (qt+1,0) runs qproj(qt+2).
                if p == 0:
                    deferred = oproj_pieces(qt - 1) if qt > 0 else qproj_pieces(1)
                else:
                    deferred = qproj_pieces(qt + 2) if qt + 2 < NT else []

            for fn in oproj_pieces(NT - 1):
                fn()

    nc.finalize()
    return nc


@functools.lru_cache(maxsize=2)
def _built(apply_mask: bool):
    return _build_nc(apply_mask)


def kernel(input, mask, q_w, q_b, k_w, k_b, v_w, v_b, o_w, o_b):
    global LAST_EXEC_NS, LAST_RESULTS
    input = np.asarray(input, dtype=np.float32)
    mask = np.asarray(mask)
    apply_mask = not bool(np.all(mask != 0))
    nc = _built(apply_mask)

    qw = (np.asarray(q_w, np.float32) / 8.0).astype(BF16)
    kw = np.asarray(k_w, np.float32).astype(BF16)
    vw = np.asarray(v_w, np.float32).astype(BF16)
    ow = np.asarray(o_w, np.float32).astype(BF16)
    qb = np.asarray(q_b, np.float32) / 8.0
    kb = np.asarray(k_b, np.float32)
    ob = (
        np.asarray(o_b, np.float64)
        + np.asarray(o_w, np.float64) @ np.asarray(v_b, np.float64)
    ).astype(np.float32)

    in_maps = []
    for c in range(N_CORES):
        b, r = divmod(c, 4)
        own = slice(OWN * r, OWN * (r + 1))
        m01 = (mask[b] != 0).astype(np.float32)
        in_maps.append(
            {
                "xT": np.ascontiguousarray(input[b].T.astype(BF16)),
                "wq": np.ascontiguousarray(qw[own, :].T),
                "wk": np.ascontiguousarray(kw[own, :].T),
                "wv": np.ascontiguousarray(vw[own, :].T),
                "wo": np.ascontiguousarray(ow[own, :].T),
                "qb2": np.ascontiguousarray(qb[own].reshape(2, 128).T),
                "kb2": np.ascontiguousarray(kb[own].reshape(2, 128).T),
                "ob2": np.ascontiguousarray(ob[own].reshape(2, 128).T),
                "mask01": np.ascontiguousarray(m01.reshape(KT, 128).T),
            }
        )

    trace = os.environ.get("KERNEL_TRACE", "0") == "1"
    res = run_bass_kernel_spmd(
        nc,
        in_maps,
        core_ids=list(range(N_CORES)),
        trace=trace,
        trace_cores=list(range(N_CORES)) if trace else None,
        stitch_traces=False,
    )
    LAST_EXEC_NS = res.exec_time_ns
    LAST_RESULTS = res

    out = np.empty((BS, L, D), dtype=np.float32)
    for c in range(N_CORES):
        b, r = divmod(c, 4)
        out[b, :, OWN * r : OWN * (r + 1)] = res.results[c]["outT"].T.astype(
            np.float32
        )
    return out
